# revision 7
# baseline (speedup 1.0000x reference)
"""Trainium2 Bass kernel: out = 2 * cummax_W(cummax_H(x)) for x [16,256,128,128] f32.

Precision: gate is rel_err < 2e-2; device works on xb = bf16(2*x) (host downcast;
x2 folded -- exact since max/x2 commute and bf16*2 is exact). Only error is the
input rounding (~2^-9 relative).

DVE scan (tensor_tensor_scan) runs at 2 cyc/elem regardless of dtype -- it is
the kernel bottleneck. The W pass uses a pair-trick to cut scanned elements in
half: with W pre-split into even|odd blocks (host-side), a 0.5 cyc/elem TT-max
combine builds pair maxes m, a half-length scan produces pair-prefix Z, and a
0.5 cyc/elem TT-max fix rebuilds even outputs; odd outputs are Z itself.
All combine/fix APs are laid out to keep the DVE 2x_1P mode (even strides,
4B-aligned starts, contiguous innermost runs); the scan output is written at
+1 into a padded 66-per-slice layout so the fix's shifted window starts on an
even element and sees NEG at each slice boundary.

Per core (2 batches = 512 slices), per supertile of g=16 slices:
  1. DMA load xt [p=h, f=(s, Eblk|Oblk)] bf16 (4KB descriptors; host h-major).
  2. DVE combine m=max(E,O) into padded mtW (stride-66 2D out, 2x mode).
  3. DVE segmented scan (bias NEG at pads) -> ztW at offset +1.
  4. DVE fix rtW = max(ztW window, E) (all 2x mode).
  5. PE: per slice two bf16 half-transposes [128,64]->[64,128] PSUM bf16:
     rtW (even w) -> partitions 0..63, ztW Z-run (odd w) -> partitions 64..127.
  6. Scalar: PSUM -> SBUF bt copy.
  7. DVE plain segmented H-scan -> ot [p=w', f=(s,h)].
  8. DMA store contiguous (4KB descriptors); host inverse-permutes w'.
"""

from contextlib import ExitStack

import numpy as np

import concourse.bass as bass
import concourse.tile as tile
from concourse import bacc, mybir
from concourse.bass_utils import run_bass_kernel_spmd
from concourse.masks import make_identity

N_CORES = 8
B, C, H, W = 16, 256, 128, 128
S = (B // N_CORES) * C  # 512 slices per core
NEG = -3.0e38

F32 = mybir.dt.float32
BF16 = mybir.dt.bfloat16

LAST_RESULTS = None


def build_nc(n_slices: int = S, g: int = 16, bufs: int = 3) -> bass.Bass:
    nc = bacc.Bacc(None, target_bir_lowering=False)
    # input: x[h, s*W + w']  (w' = evens then odds); output: o[w', s*H + h]
    x = nc.declare_dram_parameter("x", [H, n_slices * W], BF16, isOutput=False)
    o = nc.declare_dram_parameter("o", [W, n_slices * H], BF16, isOutput=True)

    n_super = n_slices // g
    assert n_super * g == n_slices
    fw = g * W  # 2048
    mw = g * 66  # padded pair-array width (2 pads + 64 pairs per slice)

    with ExitStack() as ctx:
        tc = ctx.enter_context(tile.TileContext(nc))
        consts = ctx.enter_context(tc.tile_pool(name="consts", bufs=1))
        ident = consts.tile([128, 128], BF16)
        make_identity(nc, ident)
        # W-scan bias over the padded m layout: NEG at each slice's first pad
        bias_m = consts.tile([128, mw], BF16)
        nc.vector.memset(bias_m, 0.0)
        for gi in range(g):
            nc.vector.memset(bias_m[:, gi * 66 : gi * 66 + 1], NEG)
        # H-scan bias over natural (s,h): NEG at each slice's first element
        bias_h = consts.tile([128, fw], BF16)
        nc.vector.memset(bias_h, 0.0)
        for gi in range(g):
            nc.vector.memset(bias_h[:, gi * W : gi * W + 1], NEG)

        warm_w = consts.tile([128, 2], BF16)
        nc.vector.memset(warm_w, 1.0)

        xpool = ctx.enter_context(tc.tile_pool(name="xt", bufs=bufs))
        mpool = ctx.enter_context(tc.tile_pool(name="mt", bufs=2))
        zpool = ctx.enter_context(tc.tile_pool(name="zt", bufs=bufs))
        rpool = ctx.enter_context(tc.tile_pool(name="rt", bufs=bufs))
        opool = ctx.enter_context(tc.tile_pool(name="ot", bufs=bufs))
        pa_pool = ctx.enter_context(tc.tile_pool(name="pa", bufs=4, space="PSUM"))
        pw_pool = ctx.enter_context(tc.tile_pool(name="pw", bufs=1, space="PSUM"))

        xv = x.ap()
        ov = o.ap()

        for t in range(n_super):
            xt = xpool.tile([128, fw], BF16, tag="xt")
            nc.sync.dma_start(out=xt[:], in_=xv[:, t * fw : (t + 1) * fw])
            xts = xt[:].rearrange("p (s e) -> p s e", s=g)  # e=128: E|O

            # pair combine into padded m layout (2 pads + 64 per slice)
            mt = mpool.tile([128, mw], BF16, tag="mt")
            mts = mt[:].rearrange("p (s e) -> p s e", s=g)  # e=66
            nc.vector.memset(mts[:, :, 0:2], NEG)
            nc.vector.tensor_tensor(
                mts[:, :, 2:66],
                xts[:, :, 0:64],
                xts[:, :, 64:128],
                mybir.AluOpType.max,
            )
            # segmented pair scan, written at +1 (so Z_k sits at 66s+3+k)
            zt = zpool.tile([128, mw + 2], BF16, tag="zt")
            nc.vector.tensor_tensor_scan(
                zt[:, 1 : mw + 1], bias_m[:], mt[:], 0.0,
                mybir.AluOpType.add, mybir.AluOpType.max,
            )
            zts = zt[:, : mw].rearrange("p (s e) -> p s e", s=g)  # e=66
            # fix: even outputs R_2k = max(Z_{k-1}, E_k); window starts even
            rt = rpool.tile([128, g * 64], BF16, tag="rt")
            rts = rt[:].rearrange("p (s e) -> p s e", s=g)  # e=64
            nc.vector.tensor_tensor(
                rts[:],
                zts[:, :, 2:66],
                xts[:, :, 0:64],
                mybir.AluOpType.max,
            )
            # transposes: evens from rt, odds (Z) from zt
            ot = opool.tile([128, fw], BF16, tag="ot")
            for hb in range(2):
                pa = pa_pool.tile([128, 1024], BF16, tag="pa")
                for j in range(8):
                    s = hb * 8 + j
                    nc.tensor.transpose(
                        pa[0:64, j * 128 : (j + 1) * 128],
                        rt[:, s * 64 : (s + 1) * 64],
                        ident[:],
                    )
                    nc.tensor.transpose(
                        pa[64:128, j * 128 : (j + 1) * 128],
                        zt[:, 66 * s + 3 : 66 * s + 67],
                        ident[:],
                    )
                    # tiny real matmul: transpose-mode doesn't count as
                    # PE-busy for the clock gate; this holds full pstate
                    if j % 4 == 1:
                        pw = pw_pool.tile([2, 2], F32, tag="pw")
                        nc.tensor.matmul(pw, warm_w, warm_w)
                # plain segmented H-scan over (s,h), direct from PSUM
                nc.vector.tensor_tensor_scan(
                    ot[:, hb * 1024 : (hb + 1) * 1024],
                    bias_h[:, :1024],
                    pa[:],
                    0.0,
                    mybir.AluOpType.add,
                    mybir.AluOpType.max,
                )
            nc.gpsimd.dma_start(out=ov[:, t * fw : (t + 1) * fw], in_=ot[:])
    nc.finalize()
    return nc


def kernel(x: np.ndarray) -> np.ndarray:
    global LAST_RESULTS
    import ml_dtypes

    assert x.shape == (B, C, H, W)
    x2 = np.asarray(x, dtype=np.float32) * 2.0
    xb = x2.astype(ml_dtypes.bfloat16)
    # per-core slab [S,H,W] -> deinterleave W -> h-major [H, S*W]
    xs = xb.reshape(N_CORES, S, H, W)
    xd = np.concatenate([xs[..., 0::2], xs[..., 1::2]], axis=-1)  # w' order
    in_maps = [
        {"x": np.ascontiguousarray(xd[i].transpose(1, 0, 2)).reshape(H, S * W)}
        for i in range(N_CORES)
    ]
    nc = build_nc(S, g=16, bufs=3)
    res = run_bass_kernel_spmd(nc, in_maps, core_ids=list(range(N_CORES)))
    LAST_RESULTS = res
    # o is [w'=128, S*H]: partitions 0..63 -> w=2k, 64..127 -> w=2k+1
    out = np.empty((N_CORES, S, H, W), dtype=np.float32)
    for i in range(N_CORES):
        oi = np.asarray(res.results[i]["o"]).reshape(W, S, H).transpose(1, 2, 0)
        oi = oi.astype(np.float32)  # [S, H, w']
        out[i, :, :, 0::2] = oi[:, :, 0:64]
        out[i, :, :, 1::2] = oi[:, :, 64:128]
    return out.reshape(B, C, H, W)


# revision 8
# speedup vs baseline: 1.3457x; 1.3457x over previous
"""Trainium2 Bass kernel: out = 2 * cummax_W(cummax_H(x)) for x [16,256,128,128] f32.

Precision: gate is rel_err < 2e-2; device works on xb = bf16(2*x) (host
downcast; x2 folded into the input -- exact since max/x2 commute and bf16*2 is
exact). Only error is the input rounding (~2^-9 relative).

Per core (2 batches = 512 (b,c) slices), per supertile of g=16 slices:
  1. DMA load xt [p=h, f=(s,w)] bf16, host pre-permuted h-major so each
     partition reads one 4KB contiguous run.
  2. DVE W-scan: segmented cummax via tensor_tensor_scan (bias = NEG at each
     slice's first column resets the running max).
  3. PE transposes each slice [p=h,f=w] -> PSUM bf16 [p=w,f=h] (bf16 stays
     bf16 in transpose mode; 8 slices per PSUM bank tile).
  4. DVE H-scan directly from PSUM (PSUM-src scan runs at the same 2 cyc/elem
     as SBUF) -> ot [p=w, f=(s,h)].
  5. DMA store w-major contiguous (4KB descriptors); host inverse-permutes
     and upcasts.
"""

from contextlib import ExitStack

import numpy as np

import concourse.bass as bass
import concourse.tile as tile
from concourse import bacc, mybir
from concourse.bass_utils import run_bass_kernel_spmd
from concourse.masks import make_identity

N_CORES = 8
B, C, H, W = 16, 256, 128, 128
S = (B // N_CORES) * C  # 512 slices per core
NEG = -3.0e38

F32 = mybir.dt.float32
BF16 = mybir.dt.bfloat16

LAST_RESULTS = None


def build_nc(n_slices: int = S, g: int = 16, bufs: int = 4, taper: int = 2) -> bass.Bass:
    nc = bacc.Bacc(None, target_bir_lowering=False)
    # h-major input: x[h, s*W + w]; w-major output: o[w, s*H + h]
    x = nc.declare_dram_parameter("x", [H, n_slices * W], BF16, isOutput=False)
    o = nc.declare_dram_parameter("o", [W, n_slices * H], BF16, isOutput=True)

    # schedule: small supertiles at the ends for faster pipeline fill/drain
    gs = g // 2
    chunks = []
    pos = 0
    for _ in range(taper):
        chunks.append((pos, gs))
        pos += gs
    tail = n_slices - taper * gs
    while pos < tail:
        chunks.append((pos, g))
        pos += g
    for _ in range(taper):
        chunks.append((pos, gs))
        pos += gs
    assert pos == n_slices

    with ExitStack() as ctx:
        tc = ctx.enter_context(tile.TileContext(nc))
        consts = ctx.enter_context(tc.tile_pool(name="consts", bufs=1))
        ident = consts.tile([128, 128], BF16)
        make_identity(nc, ident)
        # Segmented-scan bias: 0 everywhere, NEG at each slice's first elem.
        bias = consts.tile([128, g * W], BF16)
        nc.vector.memset(bias, 0.0)
        for gi in range(g):
            nc.vector.memset(bias[:, gi * W : gi * W + 1], NEG)

        xpool = ctx.enter_context(tc.tile_pool(name="xt", bufs=bufs))
        apool = ctx.enter_context(tc.tile_pool(name="at", bufs=bufs))
        opool = ctx.enter_context(tc.tile_pool(name="ot", bufs=bufs))
        pa_pool = ctx.enter_context(tc.tile_pool(name="pa", bufs=4, space="PSUM"))

        xv = x.ap()
        ov = o.ap()

        for s0, gc in chunks:
            fw = gc * W
            xt = xpool.tile([128, fw], BF16, tag="xt")
            nc.sync.dma_start(out=xt[:], in_=xv[:, s0 * W : s0 * W + fw])
            # cummax along W within each slice (segmented over gc slices)
            at = apool.tile([128, fw], BF16, tag="at")
            nc.vector.tensor_tensor_scan(
                at[:], bias[:, :fw], xt[:], 0.0,
                mybir.AluOpType.add, mybir.AluOpType.max,
            )
            # Transpose slices into PSUM (8 per bank tile), H-scan from PSUM
            ot = opool.tile([128, fw], BF16, tag="ot")
            for hb in range(gc // 8):
                pa = pa_pool.tile([128, 1024], BF16, tag="pa")
                for j in range(8):
                    s = hb * 8 + j
                    nc.tensor.transpose(
                        pa[:, j * 128 : (j + 1) * 128],
                        at[:, s * 128 : (s + 1) * 128],
                        ident[:],
                    )
                nc.vector.tensor_tensor_scan(
                    ot[:, hb * 1024 : (hb + 1) * 1024],
                    bias[:, :1024],
                    pa[:],
                    0.0,
                    mybir.AluOpType.add,
                    mybir.AluOpType.max,
                )
            nc.gpsimd.dma_start(out=ov[:, s0 * H : s0 * H + fw], in_=ot[:])
    nc.finalize()
    return nc


def kernel(x: np.ndarray) -> np.ndarray:
    global LAST_RESULTS
    import ml_dtypes

    assert x.shape == (B, C, H, W)
    xb = (np.asarray(x, dtype=np.float32) * 2.0).astype(ml_dtypes.bfloat16)
    xs = xb.reshape(N_CORES, S, H, W)
    in_maps = [
        {"x": np.ascontiguousarray(xs[i].transpose(1, 0, 2)).reshape(H, S * W)}
        for i in range(N_CORES)
    ]
    nc = build_nc(S, g=16, bufs=4, taper=2)
    res = run_bass_kernel_spmd(nc, in_maps, core_ids=list(range(N_CORES)))
    LAST_RESULTS = res
    # o is [W, S*H] w-major; out[s,h,w] = o[w, s*H + h]
    parts = []
    for i in range(N_CORES):
        oi = np.asarray(res.results[i]["o"]).reshape(W, S, H)
        parts.append(oi.transpose(1, 2, 0))
    out = np.stack(parts).reshape(B, C, H, W)
    return out.astype(np.float32)


# revision 11
# speedup vs baseline: 1.4570x; 1.0827x over previous
"""Trainium2 Bass kernel: out = 2 * cummax_W(cummax_H(x)) for x [16,256,128,128] f32.

Precision: gate is rel_err < 2e-2; device works on xb = bf16(2*x) (host
downcast; x2 folded into the input -- exact since max/x2 commute and bf16*2 is
exact). Only error is the input rounding (~2^-9 relative).

The DVE scan (tensor_tensor_scan) runs at 2 cyc/elem and is the bottleneck;
TT-max on aligned contiguous bf16 runs at 0.5 cyc/elem. The H pass therefore
uses a pair-trick: pair-combine adjacent h rows (0.5), scan only the pair
maxima (half the elements), and rebuild even-h outputs with one more TT-max
(0.5) -- odd-h outputs are the scan result itself. The h-even/odd split is
produced by the scalar engine while staging PSUM->SBUF (strided reads there
are off the critical path). All DVE combine/fix APs keep the 2x_1P mode:
even strides, 4B-aligned starts, contiguous runs; the scan output lands at +1
in a 66-per-slice padded layout so the fix window starts even and sees NEG at
slice boundaries.

Per core (512 slices), per supertile of g=16 slices:
  1. DMA load xt [p=h, f=(s,w)] bf16 (host h-major; 4KB descriptors).
  2. DVE W-scan: segmented cummax (bias NEG at each slice's first column).
  3. PE transposes -> PSUM bf16 [p=w, f=(s,h)], 8 slices per bank tile.
  4. Scalar: deinterleaved staging PSUM->SBUF: btE (h even), btO (h odd).
  5. DVE: combine m=max(btE,btO) into padded mt; segmented pair scan -> zt@+1;
     fix rtH = max(zt window, btE).
  6. Stores: rtH (h-even results) and zt Z-runs (h-odd results) to a
     [w, s, hE|hO] DRAM layout; host re-interleaves and upcasts.
"""

from contextlib import ExitStack

import numpy as np

import concourse.bass as bass
import concourse.tile as tile
from concourse import bacc, mybir
from concourse.bass_utils import run_bass_kernel_spmd
from concourse.masks import make_identity

N_CORES = 8
B, C, H, W = 16, 256, 128, 128
S = (B // N_CORES) * C  # 512 slices per core
NEG = -3.0e38

F32 = mybir.dt.float32
BF16 = mybir.dt.bfloat16

LAST_RESULTS = None


def build_nc(n_slices: int = S, g: int = 16, bufs: int = 4, taper: int = 2) -> bass.Bass:
    nc = bacc.Bacc(None, target_bir_lowering=False)
    # h-major input: x[h, s*W + w]; output o[w, s*128 + (hE|hO)]
    x = nc.declare_dram_parameter("x", [H, n_slices * W], BF16, isOutput=False)
    o = nc.declare_dram_parameter("o", [W, n_slices * H], BF16, isOutput=True)

    gs = g // 2
    chunks = []
    pos = 0
    for _ in range(taper):
        chunks.append((pos, gs))
        pos += gs
    tail = n_slices - taper * gs
    while pos < tail:
        chunks.append((pos, g))
        pos += g
    for _ in range(taper):
        chunks.append((pos, gs))
        pos += gs
    assert pos == n_slices

    with ExitStack() as ctx:
        tc = ctx.enter_context(tile.TileContext(nc))
        consts = ctx.enter_context(tc.tile_pool(name="consts", bufs=1))
        ident = consts.tile([128, 128], BF16)
        make_identity(nc, ident)
        # W-scan bias over (s,w): NEG at each slice's first column
        bias = consts.tile([128, g * W], BF16)
        nc.vector.memset(bias, 0.0)
        for gi in range(g):
            nc.vector.memset(bias[:, gi * W : gi * W + 1], NEG)
        # pair-scan bias over padded m layout: NEG at each slice's first pad
        bias_m = consts.tile([128, g * 66], BF16)
        nc.vector.memset(bias_m, 0.0)
        for gi in range(g):
            nc.vector.memset(bias_m[:, gi * 66 : gi * 66 + 1], NEG)

        xpool = ctx.enter_context(tc.tile_pool(name="xt", bufs=bufs))
        apool = ctx.enter_context(tc.tile_pool(name="at", bufs=bufs))
        epool = ctx.enter_context(tc.tile_pool(name="be", bufs=bufs))
        opool = ctx.enter_context(tc.tile_pool(name="bo", bufs=bufs))
        mpool = ctx.enter_context(tc.tile_pool(name="mt", bufs=2))
        zpool = ctx.enter_context(tc.tile_pool(name="zt", bufs=bufs))
        rpool = ctx.enter_context(tc.tile_pool(name="rt", bufs=bufs))
        pa_pool = ctx.enter_context(tc.tile_pool(name="pa", bufs=4, space="PSUM"))

        xv = x.ap()
        ov = o.ap()

        for s0, gc in chunks:
            fw = gc * W
            xt = xpool.tile([128, fw], BF16, tag="xt")
            nc.sync.dma_start(out=xt[:], in_=xv[:, s0 * W : s0 * W + fw])
            at = apool.tile([128, fw], BF16, tag="at")
            nc.vector.tensor_tensor_scan(
                at[:], bias[:, :fw], xt[:], 0.0,
                mybir.AluOpType.add, mybir.AluOpType.max,
            )
            # transposes + deinterleaved scalar staging
            hw = gc * 64
            btE = epool.tile([128, hw], BF16, tag="be")
            btO = opool.tile([128, hw], BF16, tag="bo")
            btEv = btE[:].rearrange("p (s e) -> p s e", s=gc)
            btOv = btO[:].rearrange("p (s e) -> p s e", s=gc)
            for hb in range(gc // 8):
                pa = pa_pool.tile([128, 1024], BF16, tag="pa")
                for j in range(8):
                    s = hb * 8 + j
                    nc.tensor.transpose(
                        pa[:, j * 128 : (j + 1) * 128],
                        at[:, s * 128 : (s + 1) * 128],
                        ident[:],
                    )
                pav = pa[:].rearrange("p (s hj hb) -> p s hj hb", s=8, hb=2)
                nc.scalar.copy(btEv[:, hb * 8 : (hb + 1) * 8], pav[:, :, :, 0])
                nc.scalar.copy(btOv[:, hb * 8 : (hb + 1) * 8], pav[:, :, :, 1])
            # pair combine into padded m layout
            mt = mpool.tile([128, gc * 66], BF16, tag="mt")
            mts = mt[:].rearrange("p (s e) -> p s e", s=gc)
            nc.vector.memset(mts[:, :, 0:2], NEG)
            nc.vector.tensor_tensor(
                mts[:, :, 2:66], btEv[:], btOv[:], mybir.AluOpType.max
            )
            # segmented pair scan, written at +1 (Z_k at 66s+3+k)
            zt = zpool.tile([128, gc * 66 + 4], BF16, tag="zt")
            nc.vector.tensor_tensor_scan(
                zt[:, 1 : gc * 66 + 1], bias_m[:, : gc * 66], mt[:], 0.0,
                mybir.AluOpType.add, mybir.AluOpType.max,
            )
            zts = zt[:, : gc * 66].rearrange("p (s e) -> p s e", s=gc)
            # fix: even-h outputs R_2k = max(Z_{k-1}, E_k); window starts even
            rt = rpool.tile([128, hw], BF16, tag="rt")
            rts = rt[:].rearrange("p (s e) -> p s e", s=gc)
            nc.vector.tensor_tensor(
                rts[:], zts[:, :, 2:66], btEv[:], mybir.AluOpType.max
            )
            # stores: evens from rt, odds (Z runs) from zt
            ovv = ov[:, s0 * H : s0 * H + fw].rearrange("p (s e) -> p s e", s=gc)
            nc.scalar.dma_start(out=ovv[:, :, 0:64], in_=rts[:])
            ztz = zt[:, 3 : 3 + gc * 66].rearrange("p (s e) -> p s e", s=gc)
            nc.gpsimd.dma_start(out=ovv[:, :, 64:128], in_=ztz[:, :, 0:64])
    nc.finalize()
    return nc


def kernel(x: np.ndarray) -> np.ndarray:
    global LAST_RESULTS
    import ml_dtypes

    assert x.shape == (B, C, H, W)
    xb = (np.asarray(x, dtype=np.float32) * 2.0).astype(ml_dtypes.bfloat16)
    xs = xb.reshape(N_CORES, S, H, W)
    in_maps = [
        {"x": np.ascontiguousarray(xs[i].transpose(1, 0, 2)).reshape(H, S * W)}
        for i in range(N_CORES)
    ]
    nc = build_nc(S, g=16, bufs=4, taper=2)
    res = run_bass_kernel_spmd(nc, in_maps, core_ids=list(range(N_CORES)))
    LAST_RESULTS = res
    # o[w, s*128 + (hE|hO)]: out[s, 2j+b, w] = o[w, s, b, j]
    out = np.empty((N_CORES, S, H, W), dtype=np.float32)
    for i in range(N_CORES):
        oi = np.asarray(res.results[i]["o"]).reshape(W, S, 2, 64).astype(np.float32)
        out[i, :, 0::2, :] = oi[:, :, 0, :].transpose(1, 2, 0)
        out[i, :, 1::2, :] = oi[:, :, 1, :].transpose(1, 2, 0)
    return out.reshape(B, C, H, W)


# revision 15
# speedup vs baseline: 1.4638x; 1.0047x over previous
"""Trainium2 Bass kernel: out = 2 * cummax_W(cummax_H(x)) for x [16,256,128,128] f32.

Precision: gate is rel_err < 2e-2; device works on xb = bf16(2*x) (host
downcast; x2 folded into the input -- exact since max/x2 commute and bf16*2 is
exact). Only error is the input rounding (~2^-9 relative).

The DVE scan (tensor_tensor_scan) runs at 2 cyc/elem and is the bottleneck;
TT-max on aligned contiguous bf16 runs at 0.5 cyc/elem. The H pass therefore
uses a pair-trick: pair-combine adjacent h rows (0.5), scan only the pair
maxima (half the elements), and rebuild even-h outputs with one more TT-max
(0.5) -- odd-h outputs are the scan result itself. The h-even/odd split is
produced by the scalar engine while staging PSUM->SBUF (strided reads there
are off the critical path). All DVE combine/fix APs keep the 2x_1P mode:
even strides, 4B-aligned starts, contiguous runs; the scan output lands at +1
in a 66-per-slice padded layout so the fix window starts even and sees NEG at
slice boundaries.

Per core (512 slices), per supertile of g=16 slices:
  1. DMA load xt [p=h, f=(s,w)] bf16 (host h-major; 4KB descriptors).
  2. DVE W-scan: segmented cummax (bias NEG at each slice's first column).
  3. PE transposes -> PSUM bf16 [p=w, f=(s,h)], 8 slices per bank tile.
  4. Scalar: deinterleaved staging PSUM->SBUF: btE (h even), btO (h odd).
  5. DVE: combine m=max(btE,btO) into padded mt; segmented pair scan -> zt@+1;
     fix rtH = max(zt window, btE).
  6. Stores: rtH (h-even results) and zt Z-runs (h-odd results) to a
     [w, s, hE|hO] DRAM layout; host re-interleaves and upcasts.
"""

from contextlib import ExitStack

import numpy as np

import concourse.bass as bass
import concourse.tile as tile
from concourse import bacc, mybir
from concourse.bass_utils import run_bass_kernel_spmd
from concourse.masks import make_identity

N_CORES = 8
B, C, H, W = 16, 256, 128, 128
S = (B // N_CORES) * C  # 512 slices per core
NEG = -3.0e38

F32 = mybir.dt.float32
BF16 = mybir.dt.bfloat16

LAST_RESULTS = None


def build_nc(n_slices: int = S, g: int = 16, bufs: int = 6, taper: int = 2) -> bass.Bass:
    nc = bacc.Bacc(None, target_bir_lowering=False)
    # h-major input: x[h, s*W + w]; output o[w, s*128 + (hE|hO)]
    x = nc.declare_dram_parameter("x", [H, n_slices * W], BF16, isOutput=False)
    o = nc.declare_dram_parameter("o", [W, n_slices * H], BF16, isOutput=True)

    gs = g // 2
    chunks = []
    pos = 0
    for _ in range(taper):
        chunks.append((pos, gs))
        pos += gs
    tail = n_slices - taper * gs
    while pos < tail:
        chunks.append((pos, g))
        pos += g
    for _ in range(taper):
        chunks.append((pos, gs))
        pos += gs
    assert pos == n_slices

    with ExitStack() as ctx:
        tc = ctx.enter_context(tile.TileContext(nc))
        consts = ctx.enter_context(tc.tile_pool(name="consts", bufs=1))
        ident = consts.tile([128, 128], BF16)
        make_identity(nc, ident)
        # W-scan bias over (s,w): NEG at each slice's first column
        bias = consts.tile([128, g * W], BF16)
        nc.vector.memset(bias, 0.0)
        for gi in range(g):
            nc.vector.memset(bias[:, gi * W : gi * W + 1], NEG)
        # pair-scan bias over padded m layout: NEG at each slice's first pad
        bias_m = consts.tile([128, g * 66], BF16)
        nc.vector.memset(bias_m, 0.0)
        for gi in range(g):
            nc.vector.memset(bias_m[:, gi * 66 : gi * 66 + 1], NEG)

        xpool = ctx.enter_context(tc.tile_pool(name="xt", bufs=bufs))
        apool = ctx.enter_context(tc.tile_pool(name="at", bufs=bufs))
        epool = ctx.enter_context(tc.tile_pool(name="be", bufs=bufs))
        opool = ctx.enter_context(tc.tile_pool(name="bo", bufs=bufs))
        mpool = ctx.enter_context(tc.tile_pool(name="mt", bufs=2))
        zpool = ctx.enter_context(tc.tile_pool(name="zt", bufs=bufs))
        rpool = ctx.enter_context(tc.tile_pool(name="rt", bufs=bufs))
        pa_pool = ctx.enter_context(tc.tile_pool(name="pa", bufs=6, space="PSUM"))

        xv = x.ap()
        ov = o.ap()

        for ci, (s0, gc) in enumerate(chunks):
            fw = gc * W
            xt = xpool.tile([128, fw], BF16, tag="xt")
            nc.sync.dma_start(out=xt[:], in_=xv[:, s0 * W : s0 * W + fw])
            at = apool.tile([128, fw], BF16, tag="at")
            nc.vector.tensor_tensor_scan(
                at[:], bias[:, :fw], xt[:], 0.0,
                mybir.AluOpType.add, mybir.AluOpType.max,
            )
            # transposes + deinterleaved scalar staging
            hw = gc * 64
            btE = epool.tile([128, hw], BF16, tag="be")
            btO = opool.tile([128, hw], BF16, tag="bo")
            btEv = btE[:].rearrange("p (s e) -> p s e", s=gc)
            btOv = btO[:].rearrange("p (s e) -> p s e", s=gc)
            for hb in range(gc // 8):
                pa = pa_pool.tile([128, 1024], BF16, tag="pa")
                for j in range(8):
                    s = hb * 8 + j
                    nc.tensor.transpose(
                        pa[:, j * 128 : (j + 1) * 128],
                        at[:, s * 128 : (s + 1) * 128],
                        ident[:],
                    )
                pav = pa[:].rearrange("p (s hj hb) -> p s hj hb", s=8, hb=2)
                nc.scalar.copy(btEv[:, hb * 8 : (hb + 1) * 8], pav[:, :, :, 0])
                nc.scalar.copy(btOv[:, hb * 8 : (hb + 1) * 8], pav[:, :, :, 1])
            # pair combine into padded m layout (full-width tile so the NEG
            # pads persist across the pool's buffer rotation)
            mt = mpool.tile([128, g * 66], BF16, tag="mt")
            mts = mt[:, : gc * 66].rearrange("p (s e) -> p s e", s=gc)
            if ci < 2:
                mfull = mt[:].rearrange("p (s e) -> p s e", s=g)
                nc.vector.memset(mfull[:, :, 0:2], NEG)
            nc.vector.tensor_tensor(
                mts[:, :, 2:66], btEv[:], btOv[:], mybir.AluOpType.max
            )
            # segmented pair scan, written at +1 (Z_k at 66s+3+k)
            zt = zpool.tile([128, gc * 66 + 4], BF16, tag="zt")
            nc.vector.tensor_tensor_scan(
                zt[:, 1 : gc * 66 + 1], bias_m[:, : gc * 66], mt[:, : gc * 66], 0.0,
                mybir.AluOpType.add, mybir.AluOpType.max,
            )
            zts = zt[:, : gc * 66].rearrange("p (s e) -> p s e", s=gc)
            # fix: even-h outputs R_2k = max(Z_{k-1}, E_k); window starts even
            rt = rpool.tile([128, hw], BF16, tag="rt")
            rts = rt[:].rearrange("p (s e) -> p s e", s=gc)
            nc.vector.tensor_tensor(
                rts[:], zts[:, :, 2:66], btEv[:], mybir.AluOpType.max
            )
            # stores: evens from rt, odds (Z runs) from zt
            ovv = ov[:, s0 * H : s0 * H + fw].rearrange("p (s e) -> p s e", s=gc)
            nc.scalar.dma_start(out=ovv[:, :, 0:64], in_=rts[:])
            ztz = zt[:, 3 : 3 + gc * 66].rearrange("p (s e) -> p s e", s=gc)
            nc.gpsimd.dma_start(out=ovv[:, :, 64:128], in_=ztz[:, :, 0:64])
    nc.finalize()
    return nc


def kernel(x: np.ndarray) -> np.ndarray:
    global LAST_RESULTS
    import ml_dtypes

    assert x.shape == (B, C, H, W)
    xb = (np.asarray(x, dtype=np.float32) * 2.0).astype(ml_dtypes.bfloat16)
    xs = xb.reshape(N_CORES, S, H, W)
    in_maps = [
        {"x": np.ascontiguousarray(xs[i].transpose(1, 0, 2)).reshape(H, S * W)}
        for i in range(N_CORES)
    ]
    nc = build_nc(S, g=16, bufs=6, taper=2)
    res = run_bass_kernel_spmd(nc, in_maps, core_ids=list(range(N_CORES)))
    LAST_RESULTS = res
    # o[w, s*128 + (hE|hO)]: out[s, 2j+b, w] = o[w, s, b, j]
    out = np.empty((N_CORES, S, H, W), dtype=np.float32)
    for i in range(N_CORES):
        oi = np.asarray(res.results[i]["o"]).reshape(W, S, 2, 64).astype(np.float32)
        out[i, :, 0::2, :] = oi[:, :, 0, :].transpose(1, 2, 0)
        out[i, :, 1::2, :] = oi[:, :, 1, :].transpose(1, 2, 0)
    return out.reshape(B, C, H, W)
